# revision 1
# baseline (speedup 1.0000x reference)
"""GCN (2-layer + BN + global mean pool + sigmoid readout) on 8 TRN2 NeuronCores.

Strategy (see spec sharding_hint): destinations (nodes) sharded across the 8
cores; each core aggregates messages for its node shard.  Per layer:

  y = dinv * (X @ W)            (node-major, bf16, exchanged via AllGather)
  agg[c] = dinv[c] * sum_{e: col_e==c} y[row_e]        (+ self loop as edge)
  h = relu(BN(agg))

The gather of y[row_e] uses bulk SWDGE dma_gather instructions (int16 indices,
128-edge chunks land edge-major on partitions).  The segment-sum is a TensorE
matmul of each gathered chunk against a small one-hot selection matrix S built
on DVE (is_equal vs iota), accumulating into PSUM windows of 32 destinations.
The instruction stream is identical on all 8 cores (SPMD); all per-core
variation lives in the input data (indices, selection metadata, padding).
"""

import numpy as np
import ml_dtypes

import concourse.bacc as bacc
import concourse.bass as bass
import concourse.tile as tile
from concourse import mybir
from concourse.bass_utils import run_bass_kernel_spmd

BF16 = ml_dtypes.bfloat16
P = 128          # partitions / chunk size
WIN = 64         # dest window width (S matrix width)
BLOCK_WINS = 8   # windows per PSUM block (8*64 = 512 dests)
EPS = 1e-5


class Dims:
    def __init__(self, N=50000, E=800000, F=96, H=128, G=64, C=50,
                 ncores=8, half=32768):
        assert N % ncores == 0
        self.N, self.E, self.F, self.H = N, E, F, H
        self.G, self.C = G, C
        self.ncores = ncores
        self.shard = N // ncores
        self.half = half                      # int16 gather range split
        self.ntile = ceil_div(self.shard, P)  # node tiles per shard
        self.shard_pad = self.ntile * P       # padded shard rows in y table
        self.npad = self.shard_pad * ncores   # padded y_full rows
        self.nwin = ceil_div(self.shard, WIN)
        self.nblk = ceil_div(self.nwin, BLOCK_WINS)


def ceil_div(a, b):
    return (a + b - 1) // b


# ----------------------------------------------------------------------------
# Host planning: pure index/graph preprocessing (functions of edge_index/batch)
# ----------------------------------------------------------------------------

class Plan:
    pass


def make_plan(d: Dims, edge_index: np.ndarray, batch: np.ndarray) -> Plan:
    pl = Plan()
    N, E = d.N, d.E
    rows = np.concatenate([edge_index[0].astype(np.int64), np.arange(N)])
    cols = np.concatenate([edge_index[1].astype(np.int64), np.arange(N)])
    deg = np.bincount(cols, minlength=N).astype(np.float64)
    dinv = (1.0 / np.sqrt(np.maximum(deg, 1.0))).astype(np.float32)

    core_of = cols // d.shard
    # remap source node id to its padded position in the all-gathered y table
    pid = (rows // d.shard) * d.shard_pad + (rows % d.shard)
    # Per (core, window, half) edge lists, edges sorted by local dest.
    per_core = []
    for k in range(d.ncores):
        m = core_of == k
        r = pid[m]
        c = cols[m] - k * d.shard
        order = np.argsort(c, kind="stable")
        r, c = r[order], c[order]
        w = c // WIN
        lo = r < d.half
        lists = {}
        # bucket by (window, half) preserving dest order
        for half_id, mask in ((0, lo), (1, ~lo)):
            rw, cw, ww = r[mask], c[mask], w[mask]
            # indices where window changes
            for wi in range(d.nwin):
                sel = ww == wi
                lists[(wi, half_id)] = (rw[sel], cw[sel])
        per_core.append(lists)

    # Shared chunk schedule: R[w][half] = max over cores of ceil(count/128)
    R = np.zeros((d.nwin, 2), dtype=np.int64)
    for k in range(d.ncores):
        for (wi, hf), (rw, cw) in per_core[k].items():
            R[wi, hf] = max(R[wi, hf], ceil_div(len(rw), P))
    R = np.maximum(R, 0)
    # every window must be initialized in PSUM: ensure at least one chunk
    for wi in range(d.nwin):
        if R[wi].sum() == 0:
            R[wi, 0] = 1

    # Build the chunk stream: per block: [lo chunks (w asc)] ++ [hi chunks]
    stream = []          # list of (window, half) per chunk position
    groups = []          # (block, half, chunk_start, chunk_count)
    for b in range(d.nblk):
        wlo = b * BLOCK_WINS
        whi = min(wlo + BLOCK_WINS, d.nwin)
        for hf in (0, 1):
            g0 = len(stream)
            for wi in range(wlo, whi):
                for _ in range(R[wi, hf]):
                    stream.append((wi, hf))
            groups.append((b, hf, g0, len(stream) - g0))
    C_grid = len(stream)
    tot_slots = C_grid * P

    # start/stop flags: matmul start=True zeroes the ENTIRE 2KB PSUM strip of
    # its output partitions, so exactly one start per (block, parity strip) --
    # the first chunk in stream order touching that strip; stop on the last.
    # all-accumulate scheme: the block PSUM tile is DVE-memset to zero, every
    # matmul uses start=False (accumulate). A start=True would zero the whole
    # 2KB PSUM strip of its partitions, wiping sibling windows in the bank.
    start_flag = np.zeros(C_grid, dtype=bool)
    stop_flag = np.zeros(C_grid, dtype=bool)

    # Fill per-core slot data
    idx_all = np.zeros((d.ncores, tot_slots), dtype=np.int16)
    A_all = np.full((d.ncores, C_grid, P), 99.0, dtype=np.float32)
    # chunk positions per (window, half) in stream order:
    pos_of = {}
    for pos, key in enumerate(stream):
        pos_of.setdefault(key, []).append(pos)
    for k in range(d.ncores):
        for (wi, hf), (rw, cw) in per_core[k].items():
            n = len(rw)
            if n == 0:
                continue
            positions = pos_of[(wi, hf)]
            assert n <= len(positions) * P
            vals = rw - (d.half if hf else 0)
            crel = cw - wi * WIN
            for j, pos in enumerate(positions):
                a, bnd = j * P, min((j + 1) * P, n)
                if a >= n:
                    break
                cnt = bnd - a
                idx_all[k, pos * P: pos * P + cnt] = vals[a:bnd].astype(np.int16)
                A_all[k, pos, :cnt] = crel[a:bnd].astype(np.float32)

    # wrap idx to the [128, tot_slots//16] layout dma_gather wants:
    # slot i -> [16*c + i%16, i//16] for every q7 core c
    S16 = tot_slots // 16
    idx_wrapped = np.zeros((d.ncores, P, S16), dtype=np.int16)
    for k in range(d.ncores):
        w16 = idx_all[k].reshape(S16, 16).T  # [16, S16]
        idx_wrapped[k] = np.tile(w16, (8, 1))

    # A matrix in [128 partitions=slot%128, C_grid] layout
    A_pt = np.transpose(A_all, (0, 2, 1)).astype(BF16)  # [cores, 128, C_grid]

    # per-core node-major helper arrays
    dinv_pt = np.zeros((d.ncores, P, d.ntile), dtype=np.float32)
    pool_pt = np.zeros((d.ncores, P, d.ntile, d.G), dtype=np.float32)
    for k in range(d.ncores):
        base = k * d.shard
        for t in range(d.ntile):
            for p in range(P):
                n0 = t * P + p
                if n0 < d.shard:
                    dinv_pt[k, p, t] = dinv[base + n0]
                    pool_pt[k, p, t, batch[base + n0]] = 1.0

    cnts = np.bincount(batch, minlength=d.G).astype(np.float32)
    inv_cnt = (1.0 / np.maximum(cnts, 1.0)).reshape(d.G, 1)

    pl.R, pl.stream, pl.groups = R, stream, groups
    pl.C_grid, pl.tot_slots = C_grid, tot_slots
    pl.start_flag, pl.stop_flag = start_flag, stop_flag
    pl.idx_wrapped, pl.A_pt = idx_wrapped, A_pt
    pl.dinv_pt, pl.pool_pt, pl.inv_cnt = dinv_pt, pool_pt.reshape(d.ncores, P, -1), inv_cnt
    pl.max_lo_chunks = max(g[3] for g in groups if g[1] == 0)
    pl.max_hi_chunks = max(g[3] for g in groups if g[1] == 1)
    blk_tot = {}
    for b, hf, g0, gc in groups:
        blk_tot[b] = blk_tot.get(b, 0) + gc
    pl.max_blk_chunks = max(blk_tot.values())
    return pl


# ----------------------------------------------------------------------------
# Bass program
# ----------------------------------------------------------------------------

def build_program(d: Dims, pl: Plan, debug=False, repeat=1, ablate=()):
    nc = bacc.Bacc("TRN2", target_bir_lowering=False, debug=False,
                   num_devices=d.ncores, num_swdge_queues=4)
    f32, bf16, i16 = mybir.dt.float32, mybir.dt.bfloat16, mybir.dt.int16

    def din(name, shape, dt=f32):
        return nc.dram_tensor(name, shape, dt, kind="ExternalInput").ap()

    xt = din("xt", [d.F, d.shard])
    W1 = din("W1", [d.F, d.H])
    W2 = din("W2", [d.H, d.H], bf16)
    Wc = din("Wc", [d.H, d.C])
    g1 = din("g1", [d.H, 1])
    be1 = din("be1", [d.H, 1])
    g2 = din("g2", [d.H, 1])
    be2 = din("be2", [d.H, 1])
    idx_d = din("idx", [P, pl.tot_slots // 16], i16)
    A_d = din("A", [P, pl.C_grid], bf16)
    dinv_d = din("dinv_pt", [P, d.ntile])
    pool_d = din("pool_pt", [P, d.ntile * d.G])
    invc_d = din("inv_cnt", [d.G, 1])
    bcr_d = din("bc_rep", [d.G, d.C])
    iota_d = din("iota", [P, WIN], bf16)
    ident_d = din("ident", [P, P])
    out_d = nc.dram_tensor("out", [d.G, d.C], f32, kind="ExternalOutput").ap()
    if debug:
        dbg_agg = nc.dram_tensor("dbg_agg", [P, d.ntile * d.H], f32,
                                 kind="ExternalOutput").ap()
        dbg_h = nc.dram_tensor("dbg_h", [d.H, d.ntile * P], f32,
                               kind="ExternalOutput").ap()
        dbg_y = nc.dram_tensor("dbg_y", [d.npad, d.H], f32,
                               kind="ExternalOutput").ap()

    rg = [list(range(d.ncores))]

    with tile.TileContext(nc) as tc:
        with (
            tc.tile_pool(name="const", bufs=1) as cpool,
            tc.tile_pool(name="work", bufs=2) as wpool,
            tc.tile_pool(name="glo", bufs=3) as gpool_lo,
            tc.tile_pool(name="ghi", bufs=3) as gpool_hi,
            tc.tile_pool(name="spool", bufs=3) as spool,
            tc.tile_pool(name="big", bufs=1) as bigpool,
            tc.tile_pool(name="pseg", bufs=3, space="PSUM") as pseg,
            tc.tile_pool(name="pmm", bufs=2, space="PSUM") as pmm,
            tc.tile_pool(name="ptr", bufs=3, space="PSUM") as ptr,
            tc.tile_pool(name="dram", bufs=1, space="DRAM") as dpool,
        ):
            # ---- load constants ----
            def cload(ap, shape, dt=f32, name=None):
                t = cpool.tile(shape, dt, tag=name)
                nc.sync.dma_start(out=t[:], in_=ap)
                return t

            W1_s = cload(W1[:], [d.F, d.H], name="W1")
            W2_s = cload(W2[:], [d.H, d.H], bf16, name="W2")
            Wc_s = cload(Wc[:], [d.H, d.C], name="Wc")
            g1_s = cload(g1[:], [d.H, 1], name="g1")
            be1_s = cload(be1[:], [d.H, 1], name="be1")
            g2_s = cload(g2[:], [d.H, 1], name="g2")
            be2_s = cload(be2[:], [d.H, 1], name="be2")
            idx_s = cload(idx_d[:], [P, pl.tot_slots // 16], i16, name="idx")
            A_s = cload(A_d[:], [P, pl.C_grid], bf16, name="A")
            dinv_s = cload(dinv_d[:], [P, d.ntile], name="dinv")
            pool_s = cload(pool_d[:], [P, d.ntile * d.G], name="pool")
            invc_s = cload(invc_d[:], [d.G, 1], name="invc")
            bcr_s = cload(bcr_d[:], [d.G, d.C], name="bcr")
            iota_s = cload(iota_d[:], [P, WIN], bf16, name="iota")
            ident_s = cload(ident_d[:], [P, P], name="ident")

            # pool matrix as bf16 for matmul
            pool_bf = cpool.tile([P, d.ntile * d.G], bf16, tag="poolbf")
            nc.vector.tensor_copy(out=pool_bf[:], in_=pool_s[:])
            eps_s = cpool.tile([d.H, 1], f32, tag="eps")
            nc.vector.memset(eps_s[:], EPS)
            ident_bf = cpool.tile([P, P], bf16, tag="identbf")
            nc.vector.tensor_copy(out=ident_bf[:], in_=ident_s[:])

            # ---- internal DRAM for collectives ----
            y_own = dpool.tile([d.shard_pad, d.H], bf16)
            y_full = dpool.tile([d.npad, d.H], bf16)
            stats_in = dpool.tile([d.H, 2], f32)
            stats_out = dpool.tile([d.H, 2], f32)
            pool_in = dpool.tile([d.G, d.H], f32)
            pool_out = dpool.tile([d.G, d.H], f32)

            h_fm = None  # feature-major relu'd activations [H, shard]
            gq = [0]

            for rep in range(repeat):
              for layer in range(2):
                # ---------- y = dinv * (X @ W)  (own shard, node-major) ----
                for mt in range(ceil_div(d.shard, 512)):
                    c0 = mt * 512
                    cw = min(512, d.shard - c0)
                    nst = ceil_div(cw, P)
                    if layer == 0:
                        rhs_t = wpool.tile([d.F, 512], f32, tag="xt_t")
                        nc.sync.dma_start(out=rhs_t[:, :cw],
                                          in_=xt[:, c0:c0 + cw])
                        lhsT, rhs_ap = W1_s[:, :], rhs_t[:, :cw]
                    else:
                        lhsT, rhs_ap = W2_s[:, :], h_fm[:, c0:c0 + cw]
                    xw_ps = pmm.tile([d.H, 512], f32, tag="xw")
                    nc.tensor.matmul(out=xw_ps[:, :cw], lhsT=lhsT,
                                     rhs=rhs_ap, start=True, stop=True)
                    xw_sb = wpool.tile([d.H, 512], f32, tag="xw_sb")
                    nc.scalar.copy(out=xw_sb[:, :cw], in_=xw_ps[:, :cw])
                    # transpose 128-node subtiles; dinv scale in ACT evac
                    y_nm = wpool.tile([P, 4, d.H], bf16, tag="y_nm")
                    for st in range(nst):
                        t_global = mt * 4 + st
                        n0 = st * P
                        nw = min(P, cw - n0)
                        tr_ps = ptr.tile([P, d.H], f32, tag="ptr")
                        nc.tensor.transpose(out=tr_ps[:nw, :],
                                            in_=xw_sb[:, n0:n0 + nw],
                                            identity=ident_s[:])
                        nc.scalar.mul(out=y_nm[:nw, st, :], in_=tr_ps[:nw, :],
                                      mul=dinv_s[:nw, t_global:t_global + 1])
                    nc.sync.dma_start(
                        out=y_own[c0:c0 + nst * P, :].rearrange(
                            "(t p) f -> p t f", p=P),
                        in_=y_nm[:, :nst, :])
                if "nogather_collective" in ablate:
                    nc.sync.dma_start(out=y_full[0:d.shard_pad, :],
                                      in_=y_own[:])
                else:
                    nc.gpsimd.collective_compute(
                        "AllGather", mybir.AluOpType.bypass, replica_groups=rg,
                        ins=[y_own.opt()], outs=[y_full.opt()])

                # ---------- gather + segment matmul over blocks ----------
                agg_dm = bigpool.tile([P, d.ntile, d.H], f32, tag="agg_dm")
                if d.shard % P:
                    nc.vector.memset(agg_dm[:, d.ntile - 1, :], 0.0)
                y_lo = y_full[0:d.half, :]
                y_hi = y_full[d.half:d.npad, :]
                gi = 0
                for b in range(d.nblk):
                    wlo = b * BLOCK_WINS
                    whi = min(wlo + BLOCK_WINS, d.nwin)
                    blk_ps = pseg.tile([P, 4 * d.H], f32, tag="seg")
                    nc.vector.memset(blk_ps[:], 0.0)
                    # gather the two half-groups of this block
                    gtiles = {}
                    for hf, gpool, ysrc in ((0, gpool_lo, y_lo),
                                            (1, gpool_hi, y_hi)):
                        _, _, g0, gcnt = pl.groups[gi]
                        gi += 1
                        if gcnt == 0:
                            gtiles[hf] = (None, g0)
                            continue
                        mgc = pl.max_lo_chunks if hf == 0 else pl.max_hi_chunks
                        gt = gpool.tile([P, mgc, d.H], bf16, tag=f"g{hf}")
                        nslots = gcnt * P
                        if "contiggather" in ablate:
                            nc.sync.dma_start(
                                out=gt[:, :gcnt, :],
                                in_=y_full[0:nslots, :].rearrange(
                                    "(s p) f -> p s f", p=P))
                        elif "nodmagather" not in ablate:
                            npc = 4  # queue-parallel pieces per group
                            bnds = [gcnt * i // npc for i in range(npc + 1)]
                            for piece, (pc0, pc1) in enumerate(
                                    zip(bnds, bnds[1:])):
                                if pc1 <= pc0:
                                    continue
                                ns_p = (pc1 - pc0) * P
                                s0 = (g0 + pc0) * P
                                nc.gpsimd.dma_gather(
                                    out_ap=gt[:, pc0:pc1, :],
                                    in_ap=ysrc,
                                    idxs_ap=idx_s[:, s0 // 16:(s0 + ns_p) // 16],
                                    num_idxs=ns_p,
                                    num_idxs_reg=ns_p,
                                    elem_size=d.H,
                                    single_packet=False,
                                    queue_num=gq[0] % 4,
                                )
                                gq[0] += 1
                        gtiles[hf] = (gt, g0)
                    # build S for all chunks of this block
                    c0 = pl.groups[gi - 2][2]
                    c1 = pl.groups[gi - 1][2] + pl.groups[gi - 1][3]
                    nch = c1 - c0
                    S_t = spool.tile([P, pl.max_blk_chunks, WIN], bf16,
                                     tag="S")
                    a_b = A_s[:, c0:c1].unsqueeze(2).broadcast_to([P, nch, WIN])
                    i_b = iota_s[:].unsqueeze(1).broadcast_to([P, nch, WIN])
                    nc.vector.tensor_tensor(out=S_t[:, :nch, :], in0=a_b,
                                            in1=i_b,
                                            op=mybir.AluOpType.is_equal)
                    # matmuls
                    for pos in (() if "nosegmm" in ablate else range(c0, c1)):
                        wi, hf = pl.stream[pos]
                        gt, g0 = gtiles[hf]
                        lc = pos - g0
                        w_in_b = wi - wlo
                        pof = WIN * (w_in_b % 2)
                        fof = d.H * (w_in_b // 2)
                        nc.tensor.matmul(
                            out=blk_ps[pof:pof + WIN, fof:fof + d.H],
                            lhsT=S_t[:, pos - c0, :],
                            rhs=gt[:, lc, :],
                            start=False, stop=bool(pl.stop_flag[pos]),
                            skip_group_check=True,
                        )
                    # evacuate: dest-major agg with dinv scaling
                    for w4 in range(ceil_div((whi - wlo) * WIN, P)):
                        t_global = (BLOCK_WINS * WIN // P) * b + w4
                        nw = min(P, d.shard - t_global * P)
                        nc.scalar.mul(
                            out=agg_dm[:nw, t_global, :],
                            in_=blk_ps[:nw, w4 * d.H:(w4 + 1) * d.H],
                            mul=dinv_s[:nw, t_global:t_global + 1])

                # ---------- stats from transposed tiles (pass 1) ----------
                s1p = wpool.tile([d.H, d.ntile], f32, tag="s1p")
                s2p = wpool.tile([d.H, d.ntile], f32, tag="s2p")
                scratch = wpool.tile([d.H, P], f32, tag="scr")
                for t in range(d.ntile):
                    tr_ps = ptr.tile([d.H, P], f32, tag="ptr")
                    nc.tensor.transpose(out=tr_ps[:, :], in_=agg_dm[:, t, :],
                                        identity=ident_s[:])
                    nc.scalar.activation(
                        out=scratch[:], in_=tr_ps[:],
                        func=mybir.ActivationFunctionType.Copy,
                        accum_out=s1p[:, t:t + 1])
                    nc.scalar.activation(
                        out=scratch[:], in_=tr_ps[:],
                        func=mybir.ActivationFunctionType.Square,
                        accum_out=s2p[:, t:t + 1])
                stats_sb = wpool.tile([d.H, 2], f32, tag="stats")
                nc.vector.tensor_reduce(out=stats_sb[:, 0:1], in_=s1p[:],
                                        axis=mybir.AxisListType.X,
                                        op=mybir.AluOpType.add)
                nc.vector.tensor_reduce(out=stats_sb[:, 1:2], in_=s2p[:],
                                        axis=mybir.AxisListType.X,
                                        op=mybir.AluOpType.add)
                nc.sync.dma_start(out=stats_in[:], in_=stats_sb[:])
                if "nostatsar" in ablate:
                    nc.sync.dma_start(out=stats_out[:], in_=stats_in[:])
                else:
                    nc.gpsimd.collective_compute(
                        "AllReduce", mybir.AluOpType.add, replica_groups=rg,
                        ins=[stats_in.opt()], outs=[stats_out.opt()])
                stats_g = wpool.tile([d.H, 2], f32, tag="statsg")
                nc.sync.dma_start(out=stats_g[:], in_=stats_out[:])
                # mean/var -> scale/bias
                mv = wpool.tile([d.H, 6], f32, tag="mv")
                inv_n = 1.0 / d.N
                nc.vector.tensor_scalar(out=mv[:, 0:1], in0=stats_g[:, 0:1],
                                        scalar1=inv_n, scalar2=None,
                                        op0=mybir.AluOpType.mult)  # mean
                nc.vector.tensor_scalar(out=mv[:, 1:2], in0=stats_g[:, 1:2],
                                        scalar1=inv_n, scalar2=None,
                                        op0=mybir.AluOpType.mult)  # E[x^2]
                nc.vector.tensor_tensor(out=mv[:, 2:3], in0=mv[:, 0:1],
                                        in1=mv[:, 0:1],
                                        op=mybir.AluOpType.mult)   # mean^2
                nc.vector.tensor_tensor(out=mv[:, 2:3], in0=mv[:, 1:2],
                                        in1=mv[:, 2:3],
                                        op=mybir.AluOpType.subtract)  # var
                nc.scalar.activation(out=mv[:, 3:4], in_=mv[:, 2:3],
                                     func=mybir.ActivationFunctionType.Sqrt,
                                     bias=eps_s[:])                # std
                nc.vector.reciprocal(out=mv[:, 4:5], in_=mv[:, 3:4])
                gg = g1_s if layer == 0 else g2_s
                bb = be1_s if layer == 0 else be2_s
                nc.vector.tensor_tensor(out=mv[:, 4:5], in0=mv[:, 4:5],
                                        in1=gg[:], op=mybir.AluOpType.mult)
                # bias = be - mean*scale
                nc.vector.tensor_tensor(out=mv[:, 5:6], in0=mv[:, 0:1],
                                        in1=mv[:, 4:5],
                                        op=mybir.AluOpType.mult)
                nc.vector.tensor_tensor(out=mv[:, 5:6], in0=bb[:],
                                        in1=mv[:, 5:6],
                                        op=mybir.AluOpType.subtract)
                if debug and layer == 0:
                    for t in range(d.ntile * d.ncores):
                        dbg_y_bf = wpool.tile([P, d.H], bf16, tag="dbgybf")
                        dbg_y_sb = wpool.tile([P, d.H], f32, tag="dbgy")
                        nc.sync.dma_start(out=dbg_y_bf[:],
                                          in_=y_full[t * P:(t + 1) * P, :])
                        nc.vector.tensor_copy(out=dbg_y_sb[:], in_=dbg_y_bf[:])
                        nc.sync.dma_start(out=dbg_y[t * P:(t + 1) * P, :],
                                          in_=dbg_y_sb[:])
                    nc.sync.dma_start(
                        out=dbg_agg[:],
                        in_=agg_dm[:].rearrange("p t f -> p (t f)"))
                h_fm = bigpool.tile([d.H, d.ntile * P], bf16, tag="h_fm")
                for t in range(d.ntile):
                    tr_ps = ptr.tile([d.H, P], f32, tag="ptr")
                    nc.tensor.transpose(out=tr_ps[:, :], in_=agg_dm[:, t, :],
                                        identity=ident_s[:])
                    nc.scalar.activation(out=h_fm[:, t * P:(t + 1) * P],
                                         in_=tr_ps[:],
                                         func=mybir.ActivationFunctionType.Relu,
                                         scale=mv[:, 4:5], bias=mv[:, 5:6])

            if debug:
                dbg_h_sb = wpool.tile([d.H, d.ntile * P], f32, tag="dbgh")
                nc.vector.tensor_copy(out=dbg_h_sb[:], in_=h_fm[:])
                nc.sync.dma_start(out=dbg_h[:], in_=dbg_h_sb[:])
            # ---------- pooling ----------
            # node-major h2 tiles via transpose, then matmul with pool matrix
            pool_ps = ptr.tile([d.G, d.H], f32, tag="ptr")
            for t in range(d.ntile):
                tr_ps = ptr.tile([P, d.H], bf16, tag="ptr")
                nc.tensor.transpose(out=tr_ps[:, :],
                                    in_=h_fm[:, t * P:(t + 1) * P],
                                    identity=ident_bf[:])
                h_dm = wpool.tile([P, d.H], bf16, tag="h_dm")
                nc.scalar.copy(out=h_dm[:], in_=tr_ps[:])
                nc.tensor.matmul(
                    out=pool_ps[:, :],
                    lhsT=pool_bf[:, t * d.G:(t + 1) * d.G],
                    rhs=h_dm[:],
                    start=(t == 0), stop=(t == d.ntile - 1))
            pool_sb = wpool.tile([d.G, d.H], f32, tag="poolsb")
            nc.vector.tensor_scalar(out=pool_sb[:], in0=pool_ps[:],
                                    scalar1=invc_s[:], scalar2=None,
                                    op0=mybir.AluOpType.mult)
            nc.sync.dma_start(out=pool_in[:], in_=pool_sb[:])
            if "nopoolar" in ablate:
                nc.sync.dma_start(out=pool_out[:], in_=pool_in[:])
            else:
                nc.gpsimd.collective_compute(
                    "AllReduce", mybir.AluOpType.add, replica_groups=rg,
                    ins=[pool_in.opt()], outs=[pool_out.opt()])
            pooled = wpool.tile([d.G, d.H], f32, tag="pooled")
            nc.sync.dma_start(out=pooled[:], in_=pool_out[:])
            # transpose pooled -> [H, G]
            pooled_t_ps = ptr.tile([d.H, d.G], f32, tag="ptr")
            nc.tensor.transpose(out=pooled_t_ps[:, :], in_=pooled[:],
                                identity=ident_s[:d.G, :d.G])
            pooled_t = wpool.tile([d.H, d.G], f32, tag="pooledtsb")
            nc.scalar.copy(out=pooled_t[:], in_=pooled_t_ps[:])
            out_ps = ptr.tile([d.G, d.C], f32, tag="ptr")
            nc.tensor.matmul(out=out_ps[:], lhsT=pooled_t[:], rhs=Wc_s[:],
                             start=True, stop=True)
            out_sb = wpool.tile([d.G, d.C], f32, tag="outsb")
            nc.vector.tensor_tensor(out=out_sb[:], in0=out_ps[:],
                                    in1=bcr_s[:], op=mybir.AluOpType.add)
            nc.scalar.activation(out=out_sb[:], in_=out_sb[:],
                                 func=mybir.ActivationFunctionType.Sigmoid)
            nc.sync.dma_start(out=out_d[:], in_=out_sb[:])

    nc.compile()
    return nc


# ----------------------------------------------------------------------------
# Entry point
# ----------------------------------------------------------------------------

def make_in_maps(d: Dims, pl: Plan, inputs):
    x = np.asarray(inputs["x"], np.float32)
    W1 = np.asarray(inputs["W1"], np.float32)
    W2 = np.asarray(inputs["W2"], np.float32)
    Wc = np.asarray(inputs["Wc"], np.float32)
    g1 = np.asarray(inputs["g1"], np.float32).reshape(d.H, 1)
    be1 = np.asarray(inputs["be1"], np.float32).reshape(d.H, 1)
    g2 = np.asarray(inputs["g2"], np.float32).reshape(d.H, 1)
    be2 = np.asarray(inputs["be2"], np.float32).reshape(d.H, 1)
    bc = np.asarray(inputs["bc"], np.float32)
    xt = np.ascontiguousarray(x.T)
    iota = np.tile(np.arange(WIN, dtype=np.float32), (P, 1)).astype(BF16)
    ident = np.eye(P, dtype=np.float32)
    bc_rep = np.tile(bc.reshape(1, d.C), (d.G, 1)).astype(np.float32)
    in_maps = []
    for k in range(d.ncores):
        in_maps.append({
            "xt": np.ascontiguousarray(xt[:, k * d.shard:(k + 1) * d.shard]),
            "W1": W1, "W2": W2.astype(BF16), "Wc": Wc,
            "g1": g1, "be1": be1, "g2": g2, "be2": be2,
            "idx": pl.idx_wrapped[k],
            "A": np.ascontiguousarray(pl.A_pt[k]),
            "dinv_pt": pl.dinv_pt[k],
            "pool_pt": pl.pool_pt[k],
            "inv_cnt": pl.inv_cnt,
            "bc_rep": bc_rep,
            "iota": iota,
            "ident": ident,
        })
    return in_maps


def kernel(**inputs) -> np.ndarray:
    d = Dims()
    edge_index = np.asarray(inputs["edge_index"], np.int64)
    batch = np.asarray(inputs["batch"], np.int64)
    pl = make_plan(d, edge_index, batch)
    nc = build_program(d, pl)
    in_maps = make_in_maps(d, pl, inputs)
    res = run_bass_kernel_spmd(nc, in_maps, core_ids=list(range(d.ncores)))
    return np.asarray(res.results[0]["out"], np.float32)



# revision 2
# speedup vs baseline: 1.0614x; 1.0614x over previous
"""GCN (2-layer + BN + global mean pool + sigmoid readout) on 8 TRN2 NeuronCores.

v2: feature-major segment aggregation.

Destinations (nodes) sharded across 8 cores. Per layer:
  y = dinv * (X @ W)     node-major via lhsT=x-tile matmuls, ACT dinv scale
  AllGather y -> y_full  (DRAM, Shared addr space)
  per dest-block of 512: dma_gather edge chunks (contiguous-packed grid),
    segment matmuls lhsT=chunk rhs=S(one-hot) -> PSUM [H, 512] feature-major,
    DVE evac -> agg_fm (dinv folded into S on host), ACT stats per block
  AllReduce stats -> BN scale/bias -> ACT relu -> h_fm (lhsT for next layer)
Pool: PE transposes of h2 + matmul with one-hot pool matrix, AllReduce, readout.
"""

import numpy as np
import ml_dtypes

import concourse.bacc as bacc
import concourse.bass as bass
import concourse.tile as tile
from concourse import mybir
from concourse.bass_utils import run_bass_kernel_spmd

BF16 = ml_dtypes.bfloat16
P = 128          # partitions / chunk size
WIN = 64         # dest window width (S matrix width)
BLOCK_WINS = 8   # windows per PSUM block (8*64 = 512 dests)
EPS = 1e-5


def ceil_div(a, b):
    return (a + b - 1) // b


class Dims:
    def __init__(self, N=50000, E=800000, F=96, H=128, G=64, C=50,
                 ncores=8, half=32768):
        assert N % ncores == 0
        self.N, self.E, self.F, self.H = N, E, F, H
        self.G, self.C = G, C
        self.ncores = ncores
        self.shard = N // ncores
        self.half = half
        self.ntile = ceil_div(self.shard, P)   # 49
        self.shard_pad = self.ntile * P        # 6272
        self.npad = self.shard_pad * ncores    # 50176
        self.nwin = ceil_div(self.shard, WIN)  # 98
        self.nblk = ceil_div(self.nwin, BLOCK_WINS)  # 13


# ----------------------------------------------------------------------------
# Host planning
# ----------------------------------------------------------------------------

class Plan:
    pass


def make_plan(d: Dims, edge_index: np.ndarray, batch: np.ndarray) -> Plan:
    pl = Plan()
    N = d.N
    rows = np.concatenate([edge_index[0].astype(np.int64), np.arange(N)])
    cols = np.concatenate([edge_index[1].astype(np.int64), np.arange(N)])
    deg = np.bincount(cols, minlength=N).astype(np.float64)
    dinv = (1.0 / np.sqrt(np.maximum(deg, 1.0))).astype(np.float32)

    core_of = cols // d.shard
    pid = (rows // d.shard) * d.shard_pad + (rows % d.shard)

    # per (core, block, half): edge lists sorted by dest
    lists = {}
    counts = np.zeros((d.ncores, d.nblk, 2), dtype=np.int64)
    for k in range(d.ncores):
        m = core_of == k
        r = pid[m]
        c = cols[m] - k * d.shard
        order = np.argsort(c, kind="stable")
        r, c = r[order], c[order]
        blk = c // (BLOCK_WINS * WIN)
        lo = r < d.half
        for b in range(d.nblk):
            for hf, hm in ((0, lo), (1, ~lo)):
                sel = (blk == b) & hm
                lists[(k, b, hf)] = (r[sel], c[sel] - b * BLOCK_WINS * WIN)
                counts[k, b, hf] = sel.sum()

    # shared chunk grid: per (block, half) group, R = ceil(max count / 128)
    R = np.zeros((d.nblk, 2), dtype=np.int64)
    for b in range(d.nblk):
        for hf in (0, 1):
            R[b, hf] = ceil_div(int(counts[:, b, hf].max()), P)

    # group slot offsets in the global stream
    group_c0 = {}
    cpos = 0
    for b in range(d.nblk):
        for hf in (0, 1):
            group_c0[(b, hf)] = cpos
            cpos += int(R[b, hf])
    C_grid = cpos
    tot_slots = C_grid * P

    # per-core slot indices + per-chunk window spans
    idx_all = np.zeros((d.ncores, tot_slots), dtype=np.int16)
    # window span per (core, chunk): min/max window index within block
    wmin = np.full((d.ncores, C_grid), 99, dtype=np.int64)
    wmax = np.full((d.ncores, C_grid), -1, dtype=np.int64)
    crel_all = np.full((d.ncores, tot_slots), -2000.0, dtype=np.float32)
    for k in range(d.ncores):
        for b in range(d.nblk):
            for hf in (0, 1):
                r, c = lists[(k, b, hf)]
                n = len(r)
                if n == 0:
                    continue
                c0 = group_c0[(b, hf)]
                s0 = c0 * P
                vals = r - (d.half if hf else 0)
                idx_all[k, s0:s0 + n] = vals.astype(np.int16)
                crel_all[k, s0:s0 + n] = c.astype(np.float32)
                w = c // WIN
                nch = ceil_div(n, P)
                for j in range(nch):
                    a, bnd = j * P, min((j + 1) * P, n)
                    wmin[k, c0 + j] = min(wmin[k, c0 + j], int(w[a]))
                    wmax[k, c0 + j] = max(wmax[k, c0 + j], int(w[bnd - 1]))

    # union window span per chunk across cores -> matmul list
    mms = []            # (chunk_pos, w_in_block, block)
    blk_mm = []         # (m0, m1) per block
    for b in range(d.nblk):
        m0 = len(mms)
        for hf in (0, 1):
            c0 = group_c0[(b, hf)]
            for j in range(int(R[b, hf])):
                pos = c0 + j
                lo_w = int(wmin[:, pos].min())
                hi_w = int(wmax[:, pos].max())
                if hi_w < 0:        # fully empty chunk (pad-only)
                    lo_w = hi_w = 0
                lo_w = min(lo_w, BLOCK_WINS - 1)
                for w in range(lo_w, hi_w + 1):
                    mms.append((pos, w, b))
        blk_mm.append((m0, len(mms)))
    n_mm = len(mms)

    # S matrices: [cores, P, n_mm, WIN] one-hot * dinv[dest], host-built
    S_mm = np.zeros((d.ncores, P, n_mm, WIN), dtype=BF16)
    slot_ids = np.arange(P)
    for k in range(d.ncores):
        base = k * d.shard
        dv = dinv[base:base + d.shard]
        for m, (pos, w, b) in enumerate(mms):
            c = crel_all[k, pos * P:(pos + 1) * P]   # block-relative dest
            cw = c - w * WIN
            valid = (cw >= 0) & (cw < WIN)
            if not valid.any():
                continue
            cols = cw[valid].astype(np.int64)
            dests = (b * BLOCK_WINS * WIN + c[valid]).astype(np.int64)
            S_mm[k, slot_ids[valid], m, cols] = dv[dests].astype(BF16)

    # idx wrap: slot i -> [16*c + i%16, i//16] replicated for 8 q7 cores
    S16 = tot_slots // 16
    idx_wrapped = np.zeros((d.ncores, P, S16), dtype=np.int16)
    for k in range(d.ncores):
        w16 = idx_all[k].reshape(S16, 16).T
        idx_wrapped[k] = np.tile(w16, (8, 1))

    # per-core node-major helpers
    dinv_pt = np.zeros((d.ncores, P, d.ntile), dtype=np.float32)
    pool_pt = np.zeros((d.ncores, P, d.ntile, d.G), dtype=np.float32)
    for k in range(d.ncores):
        base = k * d.shard
        dv = np.zeros(d.shard_pad, dtype=np.float32)
        dv[:d.shard] = dinv[base:base + d.shard]
        for t in range(d.ntile):
            dinv_pt[k, :, t] = dv[t * P:(t + 1) * P]
            for p in range(P):
                n0 = t * P + p
                if n0 < d.shard:
                    pool_pt[k, p, t, batch[base + n0]] = 1.0

    cnts = np.bincount(batch, minlength=d.G).astype(np.float32)
    inv_cnt = (1.0 / np.maximum(cnts, 1.0)).reshape(d.G, 1)

    pl.R, pl.group_c0 = R, group_c0
    pl.C_grid, pl.tot_slots, pl.n_mm = C_grid, tot_slots, n_mm
    pl.mms, pl.blk_mm = mms, blk_mm
    pl.idx_wrapped, pl.S_mm = idx_wrapped, S_mm
    pl.dinv_pt = dinv_pt
    pl.pool_pt = pool_pt.reshape(d.ncores, P, -1)
    pl.inv_cnt = inv_cnt
    pl.max_gchunks = int(R.max())
    pl.max_blk_mm = max(m1 - m0 for m0, m1 in blk_mm)
    return pl


# ----------------------------------------------------------------------------
# Bass program
# ----------------------------------------------------------------------------

def build_program(d: Dims, pl: Plan, debug=False, repeat=1, ablate=(), npc=2):
    nc = bacc.Bacc("TRN2", target_bir_lowering=False, debug=False,
                   num_devices=d.ncores, num_swdge_queues=4)
    f32, bf16, i16 = mybir.dt.float32, mybir.dt.bfloat16, mybir.dt.int16

    def din(name, shape, dt=f32):
        return nc.dram_tensor(name, shape, dt, kind="ExternalInput").ap()

    xt = din("xt", [d.F, d.shard], bf16)
    W1 = din("W1", [d.F, d.H], bf16)
    W2 = din("W2", [d.H, d.H], bf16)
    Wc = din("Wc", [d.H, d.C])
    g1 = din("g1", [d.H, 1])
    be1 = din("be1", [d.H, 1])
    g2 = din("g2", [d.H, 1])
    be2 = din("be2", [d.H, 1])
    idx_d = din("idx", [P, pl.tot_slots // 16], i16)
    S_d = din("S", [P, pl.n_mm * WIN], bf16)
    dinv_d = din("dinv_pt", [P, d.ntile])
    pool_d = din("pool_pt", [P, d.ntile * d.G])
    invc_d = din("inv_cnt", [d.G, 1])
    bcr_d = din("bc_rep", [d.G, d.C])
    ident_d = din("ident", [P, P])
    out_d = nc.dram_tensor("out", [d.G, d.C], f32, kind="ExternalOutput").ap()
    if debug:
        dbg_agg = nc.dram_tensor("dbg_agg", [d.H, d.shard_pad], f32,
                                 kind="ExternalOutput").ap()
        dbg_h = nc.dram_tensor("dbg_h", [d.H, d.shard_pad], f32,
                               kind="ExternalOutput").ap()
        dbg_y = nc.dram_tensor("dbg_y", [d.npad, d.H], f32,
                               kind="ExternalOutput").ap()

    rg = [list(range(d.ncores))]

    with tile.TileContext(nc) as tc:
        with (
            tc.tile_pool(name="const", bufs=1) as cpool,
            tc.tile_pool(name="work", bufs=2) as wpool,
            tc.tile_pool(name="glo", bufs=2) as gpool_lo,
            tc.tile_pool(name="ghi", bufs=2) as gpool_hi,
            tc.tile_pool(name="spool", bufs=2) as spool,
            tc.tile_pool(name="big", bufs=1) as bigpool,
            tc.tile_pool(name="pseg", bufs=2, space="PSUM") as pseg,
            tc.tile_pool(name="pmm", bufs=1, space="PSUM") as pmm,
            tc.tile_pool(name="dram", bufs=1, space="DRAM") as dpool,
        ):
            def cload(ap, shape, dt=f32, name=None):
                t = cpool.tile(shape, dt, tag=name)
                nc.sync.dma_start(out=t[:], in_=ap)
                return t

            W1_s = cload(W1[:], [d.F, d.H], bf16, name="W1")
            W2_s = cload(W2[:], [d.H, d.H], bf16, name="W2")
            Wc_s = cload(Wc[:], [d.H, d.C], name="Wc")
            g1_s = cload(g1[:], [d.H, 1], name="g1")
            be1_s = cload(be1[:], [d.H, 1], name="be1")
            g2_s = cload(g2[:], [d.H, 1], name="g2")
            be2_s = cload(be2[:], [d.H, 1], name="be2")
            idx_s = cload(idx_d[:], [P, pl.tot_slots // 16], i16, name="idx")
            dinv_s = cload(dinv_d[:], [P, d.ntile], name="dinv")
            pool_s = cload(pool_d[:], [P, d.ntile * d.G], name="pool")
            invc_s = cload(invc_d[:], [d.G, 1], name="invc")
            bcr_s = cload(bcr_d[:], [d.G, d.C], name="bcr")
            ident_s = cload(ident_d[:], [P, P], name="ident")

            xt_bf = cpool.tile([d.F, d.shard_pad], bf16, tag="xtbf")
            nc.vector.memset(xt_bf[:, d.shard:], 0.0)
            nc.sync.dma_start(out=xt_bf[:, :d.shard], in_=xt[:])

            pool_bf = cpool.tile([P, d.ntile * d.G], bf16, tag="poolbf")
            nc.vector.tensor_copy(out=pool_bf[:], in_=pool_s[:])
            eps_s = cpool.tile([d.H, 1], f32, tag="eps")
            nc.vector.memset(eps_s[:], EPS)
            ident_bf = cpool.tile([P, P], bf16, tag="identbf")
            nc.vector.tensor_copy(out=ident_bf[:], in_=ident_s[:])

            # internal DRAM (Shared tensors: one writer inst each)
            y_own = dpool.tile([d.shard_pad, d.H], bf16)
            y_fulls = [dpool.tile([d.npad, d.H], bf16, addr_space="Shared",
                                  name=f"y_full_{i}")
                       for i in range(2 * repeat)]
            stats_in = dpool.tile([d.H, 2], f32)
            stats_outs = [dpool.tile([d.H, 2], f32, addr_space="Shared",
                                     name=f"stats_out_{i}")
                          for i in range(2 * repeat)]
            pool_in = dpool.tile([d.G, d.H], f32)
            pool_out = dpool.tile([d.G, d.H], f32, addr_space="Shared")

            agg_fm = bigpool.tile([d.H, d.shard_pad], bf16, tag="agg_fm")
            h_fm = bigpool.tile([d.H, d.shard_pad], bf16, tag="h_fm")
            gq = [0]

            for rep in range(repeat):
              for layer in range(2):
                # ---------- y = dinv * (X @ W), node-major ----------
                lhs_src = xt_bf if layer == 0 else h_fm
                kdim = d.F if layer == 0 else d.H
                W_s = W1_s if layer == 0 else W2_s
                y_nm = wpool.tile([P, d.ntile, d.H], bf16, tag="y_nm")
                for t in range(d.ntile):
                    n0 = t * P
                    y_ps = pmm.tile([P, d.H], f32, tag="ynm", bufs=2)
                    nc.tensor.matmul(out=y_ps[:, :],
                                     lhsT=lhs_src[:kdim, n0:n0 + P],
                                     rhs=W_s[:, :], start=True, stop=True)
                    nc.scalar.mul(out=y_nm[:, t, :], in_=y_ps[:, :],
                                  mul=dinv_s[:, t:t + 1])
                nc.sync.dma_start(
                    out=y_own[:].rearrange("(t p) f -> p t f", p=P),
                    in_=y_nm[:])
                y_full = y_fulls[rep * 2 + layer]
                if "nogather_collective" in ablate:
                    nc.sync.dma_start(out=y_full[0:d.shard_pad, :],
                                      in_=y_own[:])
                else:
                    nc.gpsimd.collective_compute(
                        "AllGather", mybir.AluOpType.bypass, replica_groups=rg,
                        ins=[y_own.opt()], outs=[y_full.opt()])

                # ---------- gather + segment matmuls ----------
                y_lo = y_full[0:d.half, :]
                y_hi = y_full[d.half:d.npad, :]
                s1p = wpool.tile([d.H, d.nblk], f32, tag="s1p")
                s2p = wpool.tile([d.H, d.nblk], f32, tag="s2p")
                scratch = wpool.tile([d.H, 512], f32, tag="scr")
                for b in range(d.nblk):
                    wlo = b * BLOCK_WINS
                    whi = min(wlo + BLOCK_WINS, d.nwin)
                    bw = (whi - wlo) * WIN
                    gtiles = {}
                    for hf, gpool, ysrc in ((0, gpool_lo, y_lo),
                                            (1, gpool_hi, y_hi)):
                        gcnt = int(pl.R[b, hf])
                        c0 = pl.group_c0[(b, hf)]
                        if gcnt == 0:
                            gtiles[hf] = (None, c0)
                            continue
                        gt = gpool.tile([P, pl.max_gchunks, d.H], bf16,
                                        tag=f"g{hf}")
                        if "nodmagather" not in ablate:
                            bnds = [gcnt * i // npc for i in range(npc + 1)]
                            for pc0, pc1 in zip(bnds, bnds[1:]):
                                if pc1 <= pc0:
                                    continue
                                ns_p = (pc1 - pc0) * P
                                s0 = (c0 + pc0) * P
                                nc.gpsimd.dma_gather(
                                    out_ap=gt[:, pc0:pc1, :],
                                    in_ap=ysrc,
                                    idxs_ap=idx_s[:, s0 // 16:
                                                  (s0 + ns_p) // 16],
                                    num_idxs=ns_p,
                                    num_idxs_reg=ns_p,
                                    elem_size=d.H,
                                    single_packet=False,
                                    queue_num=gq[0] % 4,
                                )
                                gq[0] += 1
                        gtiles[hf] = (gt, c0)
                    # S for all mms of this block: DMA from DRAM (HWDGE)
                    m0, m1 = pl.blk_mm[b]
                    nmm = m1 - m0
                    S_t = spool.tile([P, pl.max_blk_mm, WIN], bf16, tag="S")
                    if "nosload" not in ablate:
                        nc.sync.dma_start(
                            out=S_t[:, :nmm, :],
                            in_=S_d[:, m0 * WIN:m1 * WIN].rearrange(
                                "p (m w) -> p m w", w=WIN))
                    # matmuls: feature-major accumulation
                    blk_ps = pseg.tile([d.H, 4 * d.H], f32, tag="seg")
                    if "nosegmm" not in ablate:
                        for m in range(m0, m1):
                            pos, w, _ = pl.mms[m]
                            hf = 0 if pos < pl.group_c0[(b, 1)] else 1
                            gt, c0g = gtiles[hf]
                            lc = pos - c0g
                            nc.tensor.matmul(
                                out=blk_ps[:, w * WIN:(w + 1) * WIN],
                                lhsT=gt[:, lc, :],
                                rhs=S_t[:, m - m0, :],
                                start=(m == m0), stop=(m == m1 - 1),
                                skip_group_check=True,
                            )
                    else:
                        nc.vector.memset(blk_ps[:], 0.0)
                    # evac to SBUF (DVE) + stats from PSUM (f32, ACT)
                    d0 = b * BLOCK_WINS * WIN
                    nc.vector.tensor_copy(out=agg_fm[:, d0:d0 + bw],
                                          in_=blk_ps[:, :bw])
                    nc.scalar.activation(
                        out=scratch[:, :bw], in_=blk_ps[:, :bw],
                        func=mybir.ActivationFunctionType.Copy,
                        accum_out=s1p[:, b:b + 1])
                    nc.scalar.activation(
                        out=scratch[:, :bw], in_=blk_ps[:, :bw],
                        func=mybir.ActivationFunctionType.Square,
                        accum_out=s2p[:, b:b + 1])

                # ---------- BN stats all-reduce ----------
                stats_sb = wpool.tile([d.H, 2], f32, tag="stats")
                nc.vector.tensor_reduce(out=stats_sb[:, 0:1], in_=s1p[:],
                                        axis=mybir.AxisListType.X,
                                        op=mybir.AluOpType.add)
                nc.vector.tensor_reduce(out=stats_sb[:, 1:2], in_=s2p[:],
                                        axis=mybir.AxisListType.X,
                                        op=mybir.AluOpType.add)
                nc.sync.dma_start(out=stats_in[:], in_=stats_sb[:])
                stats_out = stats_outs[rep * 2 + layer]
                if "nostatsar" in ablate:
                    nc.sync.dma_start(out=stats_out[:], in_=stats_in[:])
                else:
                    nc.gpsimd.collective_compute(
                        "AllReduce", mybir.AluOpType.add, replica_groups=rg,
                        ins=[stats_in.opt()], outs=[stats_out.opt()])
                stats_g = wpool.tile([d.H, 2], f32, tag="statsg")
                nc.sync.dma_start(out=stats_g[:], in_=stats_out[:])
                mv = wpool.tile([d.H, 6], f32, tag="mv")
                inv_n = 1.0 / d.N
                nc.vector.tensor_scalar(out=mv[:, 0:1], in0=stats_g[:, 0:1],
                                        scalar1=inv_n, scalar2=None,
                                        op0=mybir.AluOpType.mult)
                nc.vector.tensor_scalar(out=mv[:, 1:2], in0=stats_g[:, 1:2],
                                        scalar1=inv_n, scalar2=None,
                                        op0=mybir.AluOpType.mult)
                nc.vector.tensor_tensor(out=mv[:, 2:3], in0=mv[:, 0:1],
                                        in1=mv[:, 0:1],
                                        op=mybir.AluOpType.mult)
                nc.vector.tensor_tensor(out=mv[:, 2:3], in0=mv[:, 1:2],
                                        in1=mv[:, 2:3],
                                        op=mybir.AluOpType.subtract)
                nc.scalar.activation(out=mv[:, 3:4], in_=mv[:, 2:3],
                                     func=mybir.ActivationFunctionType.Sqrt,
                                     bias=eps_s[:])
                nc.vector.reciprocal(out=mv[:, 4:5], in_=mv[:, 3:4])
                gg = g1_s if layer == 0 else g2_s
                bb = be1_s if layer == 0 else be2_s
                nc.vector.tensor_tensor(out=mv[:, 4:5], in0=mv[:, 4:5],
                                        in1=gg[:], op=mybir.AluOpType.mult)
                nc.vector.tensor_tensor(out=mv[:, 5:6], in0=mv[:, 0:1],
                                        in1=mv[:, 4:5],
                                        op=mybir.AluOpType.mult)
                nc.vector.tensor_tensor(out=mv[:, 5:6], in0=bb[:],
                                        in1=mv[:, 5:6],
                                        op=mybir.AluOpType.subtract)
                # ---------- BN apply + relu (feature-major, per-partition) --
                nc.scalar.activation(out=h_fm[:, :], in_=agg_fm[:, :],
                                     func=mybir.ActivationFunctionType.Relu,
                                     scale=mv[:, 4:5], bias=mv[:, 5:6])
                if debug and layer == 0:
                    dbg_t = wpool.tile([d.H, d.shard_pad], f32, tag="dbga")
                    nc.vector.tensor_copy(out=dbg_t[:], in_=agg_fm[:])
                    nc.sync.dma_start(out=dbg_agg[:], in_=dbg_t[:])
                    for t in range(d.ntile * d.ncores):
                        dbg_y_bf = wpool.tile([P, d.H], bf16, tag="dbgybf")
                        dbg_y_sb = wpool.tile([P, d.H], f32, tag="dbgy")
                        nc.sync.dma_start(out=dbg_y_bf[:],
                                          in_=y_full[t * P:(t + 1) * P, :])
                        nc.vector.tensor_copy(out=dbg_y_sb[:], in_=dbg_y_bf[:])
                        nc.sync.dma_start(out=dbg_y[t * P:(t + 1) * P, :],
                                          in_=dbg_y_sb[:])

            if debug:
                dbg_h_sb = wpool.tile([d.H, d.shard_pad], f32, tag="dbgh")
                nc.vector.tensor_copy(out=dbg_h_sb[:], in_=h_fm[:])
                nc.sync.dma_start(out=dbg_h[:], in_=dbg_h_sb[:])

            # ---------- pooling ----------
            pool_ps = pmm.tile([d.G, d.H], f32, tag="poolps", bufs=1)
            for t in range(d.ntile):
                tr_ps = pmm.tile([P, d.H], bf16, tag="ptr", bufs=2)
                nc.tensor.transpose(out=tr_ps[:, :],
                                    in_=h_fm[:, t * P:(t + 1) * P],
                                    identity=ident_bf[:])
                h_dm = wpool.tile([P, d.H], bf16, tag="h_dm")
                nc.scalar.copy(out=h_dm[:], in_=tr_ps[:])
                nc.tensor.matmul(
                    out=pool_ps[:, :],
                    lhsT=pool_bf[:, t * d.G:(t + 1) * d.G],
                    rhs=h_dm[:],
                    start=(t == 0), stop=(t == d.ntile - 1))
            pool_sb = wpool.tile([d.G, d.H], f32, tag="poolsb")
            nc.vector.tensor_scalar(out=pool_sb[:], in0=pool_ps[:],
                                    scalar1=invc_s[:], scalar2=None,
                                    op0=mybir.AluOpType.mult)
            nc.sync.dma_start(out=pool_in[:], in_=pool_sb[:])
            if "nopoolar" in ablate:
                nc.sync.dma_start(out=pool_out[:], in_=pool_in[:])
            else:
                nc.gpsimd.collective_compute(
                    "AllReduce", mybir.AluOpType.add, replica_groups=rg,
                    ins=[pool_in.opt()], outs=[pool_out.opt()])
            pooled = wpool.tile([d.G, d.H], f32, tag="pooled")
            nc.sync.dma_start(out=pooled[:], in_=pool_out[:])
            pooled_t_ps = pmm.tile([d.H, d.G], f32, tag="ptr", bufs=2)
            nc.tensor.transpose(out=pooled_t_ps[:, :], in_=pooled[:],
                                identity=ident_s[:d.G, :d.G])
            pooled_t = wpool.tile([d.H, d.G], f32, tag="pooledtsb")
            nc.scalar.copy(out=pooled_t[:], in_=pooled_t_ps[:])
            out_ps = pmm.tile([d.G, d.C], f32, tag="ptr", bufs=2)
            nc.tensor.matmul(out=out_ps[:], lhsT=pooled_t[:], rhs=Wc_s[:],
                             start=True, stop=True)
            out_sb = wpool.tile([d.G, d.C], f32, tag="outsb")
            nc.vector.tensor_tensor(out=out_sb[:], in0=out_ps[:],
                                    in1=bcr_s[:], op=mybir.AluOpType.add)
            nc.scalar.activation(out=out_sb[:], in_=out_sb[:],
                                 func=mybir.ActivationFunctionType.Sigmoid)
            nc.sync.dma_start(out=out_d[:], in_=out_sb[:])

    nc.compile()
    return nc


# ----------------------------------------------------------------------------
# Entry point
# ----------------------------------------------------------------------------

def make_in_maps(d: Dims, pl: Plan, inputs):
    x = np.asarray(inputs["x"], np.float32)
    W1 = np.asarray(inputs["W1"], np.float32)
    W2 = np.asarray(inputs["W2"], np.float32)
    Wc = np.asarray(inputs["Wc"], np.float32)
    g1 = np.asarray(inputs["g1"], np.float32).reshape(d.H, 1)
    be1 = np.asarray(inputs["be1"], np.float32).reshape(d.H, 1)
    g2 = np.asarray(inputs["g2"], np.float32).reshape(d.H, 1)
    be2 = np.asarray(inputs["be2"], np.float32).reshape(d.H, 1)
    bc = np.asarray(inputs["bc"], np.float32)
    xt = np.ascontiguousarray(x.T)
    ident = np.eye(P, dtype=np.float32)
    bc_rep = np.tile(bc.reshape(1, d.C), (d.G, 1)).astype(np.float32)
    in_maps = []
    for k in range(d.ncores):
        in_maps.append({
            "xt": np.ascontiguousarray(
                xt[:, k * d.shard:(k + 1) * d.shard]).astype(BF16),
            "W1": W1.astype(BF16), "W2": W2.astype(BF16), "Wc": Wc,
            "g1": g1, "be1": be1, "g2": g2, "be2": be2,
            "idx": pl.idx_wrapped[k],
            "S": np.ascontiguousarray(pl.S_mm[k].reshape(128, -1)),
            "dinv_pt": pl.dinv_pt[k],
            "pool_pt": pl.pool_pt[k],
            "inv_cnt": pl.inv_cnt,
            "bc_rep": bc_rep,
            "ident": ident,
        })
    return in_maps


def kernel(**inputs) -> np.ndarray:
    d = Dims()
    edge_index = np.asarray(inputs["edge_index"], np.int64)
    batch = np.asarray(inputs["batch"], np.int64)
    pl = make_plan(d, edge_index, batch)
    nc = build_program(d, pl)
    in_maps = make_in_maps(d, pl, inputs)
    res = run_bass_kernel_spmd(nc, in_maps, core_ids=list(range(d.ncores)))
    return np.asarray(res.results[0]["out"], np.float32)
